# revision 17
# baseline (speedup 1.0000x reference)
"""MoE expert-parallel kernel for Trainium2 (8 NeuronCores, 1 expert/core).

Reference computation per expert e:
    h   = relu(x_e @ W1_e)               [N, DFF]
    agg[d] += h[src[k]] for dst[k]==d    (segment-sum over NE edges)
    out = agg @ W2_e                     [N, D]

Key transformations:
  1. segment_sum is linear:  (S @ h) @ W2 == S @ (h @ W2),
     where S[d, s] = #edges s->d.  Applying W2 *before* the aggregation
     halves the cost of the aggregation matmul (D < DFF).
  2. S is built on the host from edge_index (dense count matrix) so the
     gather/scatter becomes a dense matmul on the tensor engine.
  3. All matmul inputs are bf16 (fp32 PSUM accumulation).  bf16 runs at
     the same 1 row/cycle PE rate as fp32r but halves SBUF footprint and
     HBM traffic, and keeps Fast Weight Load eligible so LDWEIGHTS hides
     behind the previous matmul.  End-to-end error ~2e-3 vs the fp32
     reference (gate is 2e-2).

Device pipeline per core (expert), single fused pass:
    for each 512-token slice:  hT = relu(W1.T @ xT_slice)   (K = D)
                               m_slice = hT.T @ W2          (K = DFF)
    out = ST.T @ m                                          (K = N)
W1 (4.2 MB), W2 (4.2 MB) and all of m (8.4 MB) stay SBUF-resident, so h
never round-trips through DRAM and the PE runs back-to-back from the
first matmul to the last (216 ns/matmul steady state).  Weights are
staged just-in-time behind the first A groups; x slices and ST tiles
are double-buffered.

NOTE: the pool scoping (A/B pools closed before phase C's open) is
load-bearing for the SBUF address layout: a variant with every pool in
one scope shifted tile addresses such that weight-load and moving-
operand SBUF reads conflicted, slowing every matmul 216 -> 259 ns.
"""

import os

import numpy as np
import ml_dtypes

import concourse.bass as bass
import concourse.mybir as mybir
import concourse.tile as tile
from concourse import bacc
from concourse.bass_utils import run_bass_kernel_spmd

E, N, D, DFF = 8, 4096, 1024, 2048
P = 128
NT = N // P     # 32  token tiles
DC = D // P     # 8   d chunks (K for phase A)
FT = DFF // P   # 16  f chunks
DS = D // 512   # 2   d slices of 512
NS = N // 512   # 8   n slices of 512
SPT = 4         # token tiles per n slice

F32 = mybir.dt.float32
BF16 = mybir.dt.bfloat16
RELU = mybir.ActivationFunctionType.Relu
BF = ml_dtypes.bfloat16

_cache = {}


def _build():
    nc = bacc.Bacc()

    # xH[ns, p, dc, n'] = x[ns*512 + n', dc*128 + p]  (host-tiled: one
    # contiguous 8KB line per partition per slice)
    xH = nc.dram_tensor("xH", [NS, P, DC, 512], BF16, kind="ExternalInput")
    # W1H[ft, p, dc, f'] = W1[dc*128 + p, ft*128 + f']  (host-tiled: one
    # contiguous chunk per f-tile so W1 can be staged just-in-time)
    W1H = nc.dram_tensor("W1H", [FT, P, DC, P], BF16, kind="ExternalInput")
    W2 = nc.dram_tensor("W2", [DFF, D], BF16, kind="ExternalInput")
    # ST[nt, p, sc, n'] = S_T[sc*128 + p, nt*128 + n']  (host-tiled so each
    # phase-C load is one contiguous 8KB line per partition)
    ST = nc.dram_tensor("ST", [NT, P, NT, P], BF16, kind="ExternalInput")
    out = nc.dram_tensor("out", [N, D], F32, kind="ExternalOutput")

    with tile.TileContext(nc) as tc:
        W2r = W2.rearrange("(fc p) d -> p fc d", p=P)

        with tc.tile_pool(name="mp", bufs=1) as mp, \
             tc.tile_pool(name="psB", bufs=4, space="PSUM") as psB:
            msb = [None] * NT

            # ---------- fused phases A+B per 512-token slice ----------
            with tc.tile_pool(name="w1p", bufs=1) as w1p, \
                 tc.tile_pool(name="w2p", bufs=1) as w2p, \
                 tc.tile_pool(name="xp", bufs=2) as xp, \
                 tc.tile_pool(name="hp", bufs=2) as hp, \
                 tc.tile_pool(name="psA", bufs=3, space="PSUM") as psA:
                w1sb = w1p.tile([P, FT, DC, P], BF16, name="w1sb")
                w2sb = w2p.tile([P, FT, D], BF16, name="w2sb")
                xsbs = [
                    xp.tile([P, DC, 512], BF16, tag="xsb", name=f"xsb{i}")
                    for i in range(NS)
                ]
                # HAM warm-up: a 26-matmul accumulation group over a not-yet-
                # written region of w1sb (no new tiles -> SBUF layout
                # untouched; garbage data, result never read) keeps the PE
                # busy through the initial DMA wait so the clock-gate is at
                # 2.4 GHz when real work starts.  The w1[15] DMA lands ~30us
                # later, far after these reads retire.
                zpt = psA.tile([P, 512], F32, tag="ptA", name="ptA")
                NWU = 26
                for wu in range(NWU):
                    nc.tensor.matmul(
                        out=zpt[:],
                        lhsT=w1sb[:, FT - 1, 0],
                        rhs=w1sb[:, FT - 1, 1:5],
                        start=(wu == 0),
                        stop=(wu == NWU - 1),
                    )
                # w1[0] rides the Scalar engine's HWDGE ring so it transfers
                # in parallel with x0 on the Sync ring; x0 is split so the
                # first A matmuls release on the first half
                nc.scalar.dma_start(out=w1sb[:, 0], in_=W1H[0])
                nc.sync.dma_start(out=xsbs[0][:, 0:4], in_=xH[0, :, 0:4])
                nc.sync.dma_start(out=xsbs[0][:, 4:], in_=xH[0, :, 4:])
                for ns in range(NS):
                    # phase A: hT chunks for this slice (per-chunk tiles so
                    # phase B's deps are exact)
                    hs = [
                        hp.tile([P, 512], BF16, tag=f"h{ft}", name=f"h{ft}_{ns}")
                        for ft in range(FT)
                    ]
                    for ft in range(FT):
                        pt = psA.tile([P, 512], F32, tag="ptA", name="ptA")
                        for dc in range(DC):
                            nc.tensor.matmul(
                                out=pt[:],
                                lhsT=w1sb[:, ft, dc],
                                rhs=xsbs[ns][:, dc],
                                start=(dc == 0),
                                stop=(dc == DC - 1),
                            )
                        nc.scalar.activation(out=hs[ft][:], in_=pt[:], func=RELU)
                        if ns == 0:
                            # JIT-stage the rest of W1, then W2, behind the
                            # first slice's compute
                            if ft + 1 < FT:
                                nc.sync.dma_start(
                                    out=w1sb[:, ft + 1], in_=W1H[ft + 1]
                                )
                            nc.sync.dma_start(out=w2sb[:, ft], in_=W2r[:, ft])
                        if ft == 0 and ns + 1 < NS:
                            nc.sync.dma_start(
                                out=xsbs[ns + 1][:], in_=xH[ns + 1]
                            )
                    # phase B: m tiles for this slice (ds pair shares the
                    # stationary h block per fc step)
                    for t in range(SPT):
                        nt = ns * SPT + t
                        msb[nt] = mp.tile([P, D], BF16, tag=f"m{nt}", name=f"m{nt}")
                        pts = [
                            psB.tile([P, 512], F32, tag="ptB", name="ptB")
                            for _ in range(DS)
                        ]
                        for fc in range(FT):
                            for ds in range(DS):
                                nc.tensor.matmul(
                                    out=pts[ds][:],
                                    lhsT=hs[fc][:, t * P : (t + 1) * P],
                                    rhs=w2sb[:, fc, ds * 512 : (ds + 1) * 512],
                                    start=(fc == 0),
                                    stop=(fc == FT - 1),
                                )
                        for ds in range(DS):
                            nc.vector.tensor_copy(
                                out=msb[nt][:, ds * 512 : (ds + 1) * 512],
                                in_=pts[ds][:],
                            )

            # ---------- phase C: out = ST.T @ m ----------
            with tc.tile_pool(name="stp", bufs=2) as stp, \
                 tc.tile_pool(name="op", bufs=3) as op, \
                 tc.tile_pool(name="psC", bufs=4, space="PSUM") as psC:
                stsbs = [
                    stp.tile([P, NT, P], BF16, tag="stsb", name=f"stsb{i}")
                    for i in range(NT)
                ]
                nc.sync.dma_start(out=stsbs[0][:], in_=ST[0])
                for nt in range(NT):
                    if nt + 1 < NT:
                        nc.sync.dma_start(out=stsbs[nt + 1][:], in_=ST[nt + 1])
                    pts = [
                        psC.tile([P, 512], F32, tag="ptC", name="ptC")
                        for _ in range(DS)
                    ]
                    for sc in range(NT):
                        for ds in range(DS):
                            nc.tensor.matmul(
                                out=pts[ds][:],
                                lhsT=stsbs[nt][:, sc],
                                rhs=msb[sc][:, ds * 512 : (ds + 1) * 512],
                                start=(sc == 0),
                                stop=(sc == NT - 1),
                            )
                    for ds in range(DS):
                        osb = op.tile([P, 512], F32, tag="osb", name="osb")
                        nc.vector.tensor_copy(out=osb[:], in_=pts[ds][:])
                        eng = nc.sync if ds == 0 else nc.scalar
                        eng.dma_start(
                            out=out[
                                nt * P : (nt + 1) * P, ds * 512 : (ds + 1) * 512
                            ],
                            in_=osb[:],
                        )

    nc.compile()
    return nc


def kernel(x, W1, W2, edge_index):
    x = np.asarray(x, dtype=np.float32)
    W1 = np.asarray(W1, dtype=np.float32)
    W2 = np.asarray(W2, dtype=np.float32)
    edge_index = np.asarray(edge_index)

    # S_T[s, d] = #edges with src==s and dst==d  (so out = S_T.T @ m)
    src = edge_index[0].astype(np.int64)
    dst = edge_index[1].astype(np.int64)
    counts = np.bincount(src * N + dst, minlength=N * N)
    S_T = counts.reshape(N, N).astype(np.float32)
    # host tiling for contiguous phase-C DMA: [nt, p, sc, n']
    STH = np.ascontiguousarray(
        S_T.reshape(NT, P, NT, P).transpose(2, 1, 0, 3)
    ).astype(BF)

    if "nc" not in _cache:
        _cache["nc"] = _build()
    nc = _cache["nc"]

    in_maps = []
    for e in range(E):
        # W1H[ft, p, dc, f'] = W1[e, dc*128+p, ft*128+f']
        W1H = np.ascontiguousarray(
            W1[e].reshape(DC, P, FT, P).transpose(2, 1, 0, 3)
        ).astype(BF)
        # xH[ns, p, dc, n'] = x[e, ns*512+n', dc*128+p]
        xHe = np.ascontiguousarray(
            x[e].reshape(NS, 512, DC, P).transpose(0, 3, 2, 1)
        ).astype(BF)
        in_maps.append(
            {
                "xH": xHe,
                "W1H": W1H,
                "W2": W2[e].astype(BF),
                "ST": STH,
            }
        )

    trace = bool(int(os.environ.get("PROBLEM_TRACE", "0")))
    res = run_bass_kernel_spmd(nc, in_maps, core_ids=list(range(E)), trace=trace)
    _cache["last_results"] = res
    return np.stack([res.results[e]["out"] for e in range(E)]).astype(np.float32)


# revision 19
# speedup vs baseline: 1.0022x; 1.0022x over previous
"""MoE expert-parallel kernel for Trainium2 (8 NeuronCores, 1 expert/core).

Reference computation per expert e:
    h   = relu(x_e @ W1_e)               [N, DFF]
    agg[d] += h[src[k]] for dst[k]==d    (segment-sum over NE edges)
    out = agg @ W2_e                     [N, D]

Key transformations:
  1. segment_sum is linear:  (S @ h) @ W2 == S @ (h @ W2),
     where S[d, s] = #edges s->d.  Applying W2 *before* the aggregation
     halves the cost of the aggregation matmul (D < DFF).
  2. S is built on the host from edge_index (dense count matrix) so the
     gather/scatter becomes a dense matmul on the tensor engine.
  3. All matmul inputs are bf16 (fp32 PSUM accumulation).  bf16 runs at
     the same 1 row/cycle PE rate as fp32r but halves SBUF footprint and
     HBM traffic, and keeps Fast Weight Load eligible so LDWEIGHTS hides
     behind the previous matmul.  End-to-end error ~2e-3 vs the fp32
     reference (gate is 2e-2).

Device pipeline per core (expert), single fused pass:
    for each 512-token slice:  hT = relu(W1.T @ xT_slice)   (K = D)
                               m_slice = hT.T @ W2          (K = DFF)
    out = ST.T @ m                                          (K = N)
W1 (4.2 MB), W2 (4.2 MB) and all of m (8.4 MB) stay SBUF-resident, so h
never round-trips through DRAM and the PE runs back-to-back from the
first matmul to the last (216 ns/matmul steady state).  Weights are
staged just-in-time behind the first A groups; x slices and ST tiles
are double-buffered.

NOTE: the pool scoping (A/B pools closed before phase C's open) is
load-bearing for the SBUF address layout: a variant with every pool in
one scope shifted tile addresses such that weight-load and moving-
operand SBUF reads conflicted, slowing every matmul 216 -> 259 ns.
"""

import os

import numpy as np
import ml_dtypes

import concourse.bass as bass
import concourse.mybir as mybir
import concourse.tile as tile
from concourse import bacc
from concourse.bass_utils import run_bass_kernel_spmd

E, N, D, DFF = 8, 4096, 1024, 2048
P = 128
NT = N // P     # 32  token tiles
DC = D // P     # 8   d chunks (K for phase A)
FT = DFF // P   # 16  f chunks
DS = D // 512   # 2   d slices of 512
NS = N // 512   # 8   n slices of 512
SPT = 4         # token tiles per n slice

F32 = mybir.dt.float32
BF16 = mybir.dt.bfloat16
RELU = mybir.ActivationFunctionType.Relu
BF = ml_dtypes.bfloat16

_cache = {}


def _build():
    nc = bacc.Bacc()

    # xH[ns, p, dc, n'] = x[ns*512 + n', dc*128 + p]  (host-tiled: one
    # contiguous 8KB line per partition per slice)
    xH = nc.dram_tensor("xH", [NS, P, DC, 512], BF16, kind="ExternalInput")
    # W1H[ft, p, dc, f'] = W1[dc*128 + p, ft*128 + f']  (host-tiled: one
    # contiguous chunk per f-tile so W1 can be staged just-in-time)
    W1H = nc.dram_tensor("W1H", [FT, P, DC, P], BF16, kind="ExternalInput")
    W2 = nc.dram_tensor("W2", [DFF, D], BF16, kind="ExternalInput")
    # ST[nt, p, sc, n'] = S_T[sc*128 + p, nt*128 + n']  (host-tiled so each
    # phase-C load is one contiguous 8KB line per partition)
    ST = nc.dram_tensor("ST", [NT, P, NT, P], BF16, kind="ExternalInput")
    out = nc.dram_tensor("out", [N, D], F32, kind="ExternalOutput")

    with tile.TileContext(nc) as tc:
        W2r = W2.rearrange("(fc p) d -> p fc d", p=P)

        with tc.tile_pool(name="mp", bufs=1) as mp, \
             tc.tile_pool(name="psB", bufs=4, space="PSUM") as psB:
            msb = [None] * NT

            # ---------- fused phases A+B per 512-token slice ----------
            with tc.tile_pool(name="w1p", bufs=1) as w1p, \
                 tc.tile_pool(name="w2p", bufs=1) as w2p, \
                 tc.tile_pool(name="xp", bufs=2) as xp, \
                 tc.tile_pool(name="hp", bufs=2) as hp, \
                 tc.tile_pool(name="psA", bufs=3, space="PSUM") as psA:
                w1sb = w1p.tile([P, FT, DC, P], BF16, name="w1sb")
                w2sb = w2p.tile([P, FT, D], BF16, name="w2sb")
                xsbs = [
                    xp.tile([P, DC, 512], BF16, tag="xsb", name=f"xsb{i}")
                    for i in range(NS)
                ]
                # w1[0] rides the Scalar engine's HWDGE ring so it transfers
                # in parallel with x0 on the Sync ring; x0 is split so the
                # first A matmuls release on the first half
                nc.scalar.dma_start(out=w1sb[:, 0], in_=W1H[0])
                nc.sync.dma_start(out=xsbs[0][:, 0:4], in_=xH[0, :, 0:4])
                nc.sync.dma_start(out=xsbs[0][:, 4:], in_=xH[0, :, 4:])
                for ns in range(NS):
                    # phase A: hT chunks for this slice (per-chunk tiles so
                    # phase B's deps are exact)
                    hs = [
                        hp.tile([P, 512], BF16, tag=f"h{ft}", name=f"h{ft}_{ns}")
                        for ft in range(FT)
                    ]
                    for ft in range(FT):
                        pt = psA.tile([P, 512], F32, tag="ptA", name="ptA")
                        for dc in range(DC):
                            nc.tensor.matmul(
                                out=pt[:],
                                lhsT=w1sb[:, ft, dc],
                                rhs=xsbs[ns][:, dc],
                                start=(dc == 0),
                                stop=(dc == DC - 1),
                            )
                        nc.scalar.activation(out=hs[ft][:], in_=pt[:], func=RELU)
                        if ns == 0:
                            # JIT-stage the rest of W1, then W2, behind the
                            # first slice's compute
                            if ft + 1 < FT:
                                nc.sync.dma_start(
                                    out=w1sb[:, ft + 1], in_=W1H[ft + 1]
                                )
                            nc.sync.dma_start(out=w2sb[:, ft], in_=W2r[:, ft])
                        if ft == 0 and ns + 1 < NS:
                            nc.sync.dma_start(
                                out=xsbs[ns + 1][:], in_=xH[ns + 1]
                            )
                    # phase B: m tiles for this slice (ds pair shares the
                    # stationary h block per fc step)
                    for t in range(SPT):
                        nt = ns * SPT + t
                        msb[nt] = mp.tile([P, D], BF16, tag=f"m{nt}", name=f"m{nt}")
                        pts = [
                            psB.tile([P, 512], F32, tag="ptB", name="ptB")
                            for _ in range(DS)
                        ]
                        for fc in range(FT):
                            for ds in range(DS):
                                nc.tensor.matmul(
                                    out=pts[ds][:],
                                    lhsT=hs[fc][:, t * P : (t + 1) * P],
                                    rhs=w2sb[:, fc, ds * 512 : (ds + 1) * 512],
                                    start=(fc == 0),
                                    stop=(fc == FT - 1),
                                )
                        for ds in range(DS):
                            nc.vector.tensor_copy(
                                out=msb[nt][:, ds * 512 : (ds + 1) * 512],
                                in_=pts[ds][:],
                            )

            # ---------- phase C: out = ST.T @ m ----------
            with tc.tile_pool(name="stp", bufs=2) as stp, \
                 tc.tile_pool(name="op", bufs=3) as op, \
                 tc.tile_pool(name="psC", bufs=4, space="PSUM") as psC:
                stsbs = [
                    stp.tile([P, NT, P], BF16, tag="stsb", name=f"stsb{i}")
                    for i in range(NT)
                ]
                nc.sync.dma_start(out=stsbs[0][:], in_=ST[0])
                for nt in range(NT - 1):
                    nc.sync.dma_start(out=stsbs[nt + 1][:], in_=ST[nt + 1])
                    pts = [
                        psC.tile([P, 512], F32, tag="ptC", name="ptC")
                        for _ in range(DS)
                    ]
                    for sc in range(NT):
                        for ds in range(DS):
                            nc.tensor.matmul(
                                out=pts[ds][:],
                                lhsT=stsbs[nt][:, sc],
                                rhs=msb[sc][:, ds * 512 : (ds + 1) * 512],
                                start=(sc == 0),
                                stop=(sc == NT - 1),
                            )
                    for ds in range(DS):
                        osb = op.tile([P, 512], F32, tag="osb", name="osb")
                        nc.vector.tensor_copy(out=osb[:], in_=pts[ds][:])
                        eng = nc.sync if ds == 0 else nc.scalar
                        eng.dma_start(
                            out=out[
                                nt * P : (nt + 1) * P, ds * 512 : (ds + 1) * 512
                            ],
                            in_=osb[:],
                        )
                # last tile: four sequential quarter-width contractions, so
                # only a 256-wide cast + small write sit after the final
                # matmul (the first three chains retire under later
                # subgroups' matmuls)
                nt = NT - 1
                for q in range(4):
                    ptq = psC.tile([P, 256], F32, tag="ptC", name="ptC")
                    for sc in range(NT):
                        nc.tensor.matmul(
                            out=ptq[:],
                            lhsT=stsbs[nt][:, sc],
                            rhs=msb[sc][:, q * 256 : (q + 1) * 256],
                            start=(sc == 0),
                            stop=(sc == NT - 1),
                        )
                    osb = op.tile([P, 256], F32, tag="osb", name="osb")
                    nc.vector.tensor_copy(out=osb[:], in_=ptq[:])
                    eng = nc.sync if q % 2 == 0 else nc.scalar
                    eng.dma_start(
                        out=out[nt * P : (nt + 1) * P, q * 256 : (q + 1) * 256],
                        in_=osb[:],
                    )

    nc.compile()
    return nc


def kernel(x, W1, W2, edge_index):
    x = np.asarray(x, dtype=np.float32)
    W1 = np.asarray(W1, dtype=np.float32)
    W2 = np.asarray(W2, dtype=np.float32)
    edge_index = np.asarray(edge_index)

    # S_T[s, d] = #edges with src==s and dst==d  (so out = S_T.T @ m)
    src = edge_index[0].astype(np.int64)
    dst = edge_index[1].astype(np.int64)
    counts = np.bincount(src * N + dst, minlength=N * N)
    S_T = counts.reshape(N, N).astype(np.float32)
    # host tiling for contiguous phase-C DMA: [nt, p, sc, n']
    STH = np.ascontiguousarray(
        S_T.reshape(NT, P, NT, P).transpose(2, 1, 0, 3)
    ).astype(BF)

    if "nc" not in _cache:
        _cache["nc"] = _build()
    nc = _cache["nc"]

    in_maps = []
    for e in range(E):
        # W1H[ft, p, dc, f'] = W1[e, dc*128+p, ft*128+f']
        W1H = np.ascontiguousarray(
            W1[e].reshape(DC, P, FT, P).transpose(2, 1, 0, 3)
        ).astype(BF)
        # xH[ns, p, dc, n'] = x[e, ns*512+n', dc*128+p]
        xHe = np.ascontiguousarray(
            x[e].reshape(NS, 512, DC, P).transpose(0, 3, 2, 1)
        ).astype(BF)
        in_maps.append(
            {
                "xH": xHe,
                "W1H": W1H,
                "W2": W2[e].astype(BF),
                "ST": STH,
            }
        )

    trace = bool(int(os.environ.get("PROBLEM_TRACE", "0")))
    res = run_bass_kernel_spmd(nc, in_maps, core_ids=list(range(E)), trace=trace)
    _cache["last_results"] = res
    return np.stack([res.results[e]["out"] for e in range(E)]).astype(np.float32)


# revision 20
# speedup vs baseline: 1.0048x; 1.0026x over previous
"""MoE expert-parallel kernel for Trainium2 (8 NeuronCores, 1 expert/core).

Reference computation per expert e:
    h   = relu(x_e @ W1_e)               [N, DFF]
    agg[d] += h[src[k]] for dst[k]==d    (segment-sum over NE edges)
    out = agg @ W2_e                     [N, D]

Key transformations:
  1. segment_sum is linear:  (S @ h) @ W2 == S @ (h @ W2),
     where S[d, s] = #edges s->d.  Applying W2 *before* the aggregation
     halves the cost of the aggregation matmul (D < DFF).
  2. S is built on the host from edge_index (dense count matrix) so the
     gather/scatter becomes a dense matmul on the tensor engine.
  3. All matmul inputs are bf16 (fp32 PSUM accumulation).  bf16 runs at
     the same 1 row/cycle PE rate as fp32r but halves SBUF footprint and
     HBM traffic, and keeps Fast Weight Load eligible so LDWEIGHTS hides
     behind the previous matmul.  End-to-end error ~2e-3 vs the fp32
     reference (gate is 2e-2).

Device pipeline per core (expert), single fused pass:
    for each 512-token slice:  hT = relu(W1.T @ xT_slice)   (K = D)
                               m_slice = hT.T @ W2          (K = DFF)
    out = ST.T @ m                                          (K = N)
W1 (4.2 MB), W2 (4.2 MB) and all of m (8.4 MB) stay SBUF-resident, so h
never round-trips through DRAM and the PE runs back-to-back from the
first matmul to the last (216 ns/matmul steady state).  Weights are
staged just-in-time behind the first A groups; x slices and ST tiles
are double-buffered.

NOTE: the pool scoping (A/B pools closed before phase C's open) is
load-bearing for the SBUF address layout: a variant with every pool in
one scope shifted tile addresses such that weight-load and moving-
operand SBUF reads conflicted, slowing every matmul 216 -> 259 ns.
"""

import os

import numpy as np
import ml_dtypes

import concourse.bass as bass
import concourse.mybir as mybir
import concourse.tile as tile
from concourse import bacc
from concourse.bass_utils import run_bass_kernel_spmd

E, N, D, DFF = 8, 4096, 1024, 2048
P = 128
NT = N // P     # 32  token tiles
DC = D // P     # 8   d chunks (K for phase A)
FT = DFF // P   # 16  f chunks
DS = D // 512   # 2   d slices of 512
NS = N // 512   # 8   n slices of 512
SPT = 4         # token tiles per n slice

F32 = mybir.dt.float32
BF16 = mybir.dt.bfloat16
RELU = mybir.ActivationFunctionType.Relu
BF = ml_dtypes.bfloat16

_cache = {}


def _build():
    nc = bacc.Bacc()

    # xH[ns, p, dc, n'] = x[ns*512 + n', dc*128 + p]  (host-tiled: one
    # contiguous 8KB line per partition per slice)
    xH = nc.dram_tensor("xH", [NS, P, DC, 512], BF16, kind="ExternalInput")
    # W1H[ft, p, dc, f'] = W1[dc*128 + p, ft*128 + f']  (host-tiled: one
    # contiguous chunk per f-tile so W1 can be staged just-in-time)
    W1H = nc.dram_tensor("W1H", [FT, P, DC, P], BF16, kind="ExternalInput")
    W2 = nc.dram_tensor("W2", [DFF, D], BF16, kind="ExternalInput")
    # ST[nt, p, sc, n'] = S_T[sc*128 + p, nt*128 + n']  (host-tiled so each
    # phase-C load is one contiguous 8KB line per partition)
    ST = nc.dram_tensor("ST", [NT, P, NT, P], BF16, kind="ExternalInput")
    out = nc.dram_tensor("out", [N, D], F32, kind="ExternalOutput")

    with tile.TileContext(nc) as tc:
        W2r = W2.rearrange("(fc p) d -> p fc d", p=P)

        with tc.tile_pool(name="mp", bufs=1) as mp, \
             tc.tile_pool(name="psB", bufs=4, space="PSUM") as psB:
            msb = [None] * NT

            # ---------- fused phases A+B per 512-token slice ----------
            with tc.tile_pool(name="w1p", bufs=1) as w1p, \
                 tc.tile_pool(name="w2p", bufs=1) as w2p, \
                 tc.tile_pool(name="xp", bufs=2) as xp, \
                 tc.tile_pool(name="hp", bufs=2) as hp, \
                 tc.tile_pool(name="psA", bufs=3, space="PSUM") as psA:
                w1sb = w1p.tile([P, FT, DC, P], BF16, name="w1sb")
                w2sb = w2p.tile([P, FT, D], BF16, name="w2sb")
                xsbs = [
                    xp.tile([P, DC, 512], BF16, tag="xsb", name=f"xsb{i}")
                    for i in range(NS)
                ]
                # w1[0] rides the Scalar engine's HWDGE ring so it transfers
                # in parallel with x0 on the Sync ring; x0 is split so the
                # first A matmuls release on the first half
                nc.scalar.dma_start(out=w1sb[:, 0], in_=W1H[0])
                nc.sync.dma_start(out=xsbs[0][:, 0:4], in_=xH[0, :, 0:4])
                nc.sync.dma_start(out=xsbs[0][:, 4:], in_=xH[0, :, 4:])
                for ns in range(NS):
                    # phase A: hT chunks for this slice (per-chunk tiles so
                    # phase B's deps are exact)
                    hs = [
                        hp.tile([P, 512], BF16, tag=f"h{ft}", name=f"h{ft}_{ns}")
                        for ft in range(FT)
                    ]
                    for ft in range(FT):
                        pt = psA.tile([P, 512], F32, tag="ptA", name="ptA")
                        for dc in range(DC):
                            nc.tensor.matmul(
                                out=pt[:],
                                lhsT=w1sb[:, ft, dc],
                                rhs=xsbs[ns][:, dc],
                                start=(dc == 0),
                                stop=(dc == DC - 1),
                            )
                        nc.scalar.activation(out=hs[ft][:], in_=pt[:], func=RELU)
                        if ns == 0:
                            # JIT-stage the rest of W1, then W2, behind the
                            # first slice's compute
                            if ft + 1 < FT:
                                nc.sync.dma_start(
                                    out=w1sb[:, ft + 1], in_=W1H[ft + 1]
                                )
                            nc.sync.dma_start(out=w2sb[:, ft], in_=W2r[:, ft])
                        if ft == 0 and ns + 1 < NS:
                            nc.sync.dma_start(
                                out=xsbs[ns + 1][:], in_=xH[ns + 1]
                            )
                    # phase B: m tiles for this slice (ds pair shares the
                    # stationary h block per fc step)
                    for t in range(SPT):
                        nt = ns * SPT + t
                        msb[nt] = mp.tile([P, D], BF16, tag=f"m{nt}", name=f"m{nt}")
                        pts = [
                            psB.tile([P, 512], F32, tag="ptB", name="ptB")
                            for _ in range(DS)
                        ]
                        for fc in range(FT):
                            for ds in range(DS):
                                nc.tensor.matmul(
                                    out=pts[ds][:],
                                    lhsT=hs[fc][:, t * P : (t + 1) * P],
                                    rhs=w2sb[:, fc, ds * 512 : (ds + 1) * 512],
                                    start=(fc == 0),
                                    stop=(fc == FT - 1),
                                )
                        for ds in range(DS):
                            nc.vector.tensor_copy(
                                out=msb[nt][:, ds * 512 : (ds + 1) * 512],
                                in_=pts[ds][:],
                            )

            # ---------- phase C: out = ST.T @ m ----------
            with tc.tile_pool(name="stp", bufs=2) as stp, \
                 tc.tile_pool(name="op", bufs=3) as op, \
                 tc.tile_pool(name="psC", bufs=4, space="PSUM") as psC:
                stsbs = [
                    stp.tile([P, NT, P], BF16, tag="stsb", name=f"stsb{i}")
                    for i in range(NT)
                ]
                nc.sync.dma_start(out=stsbs[0][:], in_=ST[0])
                for nt in range(NT - 1):
                    nc.sync.dma_start(out=stsbs[nt + 1][:], in_=ST[nt + 1])
                    pts = [
                        psC.tile([P, 512], F32, tag="ptC", name="ptC")
                        for _ in range(DS)
                    ]
                    for sc in range(NT):
                        for ds in range(DS):
                            nc.tensor.matmul(
                                out=pts[ds][:],
                                lhsT=stsbs[nt][:, sc],
                                rhs=msb[sc][:, ds * 512 : (ds + 1) * 512],
                                start=(sc == 0),
                                stop=(sc == NT - 1),
                            )
                    for ds in range(DS):
                        osb = op.tile([P, 512], F32, tag="osb", name="osb")
                        nc.vector.tensor_copy(out=osb[:], in_=pts[ds][:])
                        eng = nc.sync if ds == 0 else nc.scalar
                        eng.dma_start(
                            out=out[
                                nt * P : (nt + 1) * P, ds * 512 : (ds + 1) * 512
                            ],
                            in_=osb[:],
                        )
                # last tile: four sequential quarter-width contractions, so
                # only a 256-wide cast + small write sit after the final
                # matmul (the first three chains retire under later
                # subgroups' matmuls)
                nt = NT - 1
                SPLITS = [(0, 512), (512, 256), (768, 128), (896, 128)]
                for q, (c0, w) in enumerate(SPLITS):
                    ptq = psC.tile([P, w], F32, tag="ptC", name="ptC")
                    for sc in range(NT):
                        nc.tensor.matmul(
                            out=ptq[:],
                            lhsT=stsbs[nt][:, sc],
                            rhs=msb[sc][:, c0 : c0 + w],
                            start=(sc == 0),
                            stop=(sc == NT - 1),
                        )
                    osb = op.tile([P, w], F32, tag="osb", name="osb")
                    nc.vector.tensor_copy(out=osb[:], in_=ptq[:])
                    eng = nc.sync if q % 2 == 0 else nc.scalar
                    eng.dma_start(
                        out=out[nt * P : (nt + 1) * P, c0 : c0 + w],
                        in_=osb[:],
                    )

    nc.compile()
    return nc


def kernel(x, W1, W2, edge_index):
    x = np.asarray(x, dtype=np.float32)
    W1 = np.asarray(W1, dtype=np.float32)
    W2 = np.asarray(W2, dtype=np.float32)
    edge_index = np.asarray(edge_index)

    # S_T[s, d] = #edges with src==s and dst==d  (so out = S_T.T @ m)
    src = edge_index[0].astype(np.int64)
    dst = edge_index[1].astype(np.int64)
    counts = np.bincount(src * N + dst, minlength=N * N)
    S_T = counts.reshape(N, N).astype(np.float32)
    # host tiling for contiguous phase-C DMA: [nt, p, sc, n']
    STH = np.ascontiguousarray(
        S_T.reshape(NT, P, NT, P).transpose(2, 1, 0, 3)
    ).astype(BF)

    if "nc" not in _cache:
        _cache["nc"] = _build()
    nc = _cache["nc"]

    in_maps = []
    for e in range(E):
        # W1H[ft, p, dc, f'] = W1[e, dc*128+p, ft*128+f']
        W1H = np.ascontiguousarray(
            W1[e].reshape(DC, P, FT, P).transpose(2, 1, 0, 3)
        ).astype(BF)
        # xH[ns, p, dc, n'] = x[e, ns*512+n', dc*128+p]
        xHe = np.ascontiguousarray(
            x[e].reshape(NS, 512, DC, P).transpose(0, 3, 2, 1)
        ).astype(BF)
        in_maps.append(
            {
                "xH": xHe,
                "W1H": W1H,
                "W2": W2[e].astype(BF),
                "ST": STH,
            }
        )

    trace = bool(int(os.environ.get("PROBLEM_TRACE", "0")))
    res = run_bass_kernel_spmd(nc, in_maps, core_ids=list(range(E)), trace=trace)
    _cache["last_results"] = res
    return np.stack([res.results[e]["out"] for e in range(E)]).astype(np.float32)
